# revision 13
# baseline (speedup 1.0000x reference)
"""Trainium2 Bass kernel for nn_ODEOneSeqDecoder (RK4 Kutta-3/8 neural ODE decoder).

Sharding: data-parallel over batch across 8 NeuronCores (128 batch each).
On-chip state is transposed ([features=128, batch]); the 511-step scan is fully
unrolled with hand-scheduled PE/ACT/DVE pipelines under a single-attached-wait
semaphore discipline (this walrus build rejects multi-wait instructions).

cos(3(x+b)) is computed as Sin(scale=3, bias=3b+pi/2) on the scalar engine.
The Sin table is only accurate on [-pi, pi], so persistent per-element angle
corrections c1 [32,B], c2 [64,B] (multiples of 2pi/3) are *added into the PSUM
preactivation via identity matmuls on the PE* (keeping the reduction off the
DVE critical path) and refreshed once per step on the DVE with the fp32
magic-number rounding trick. Host bootstraps c1/c2 exactly from y0.

Observation projection runs inline: one tiny matmul per step produces
[batch,3] PSUM slots (128 steps per bank), flushed to SBUF every 128 steps and
DMA'd out once. b_obs add, sigmoid, transposes, and logpt run on host.
"""
import math
import os
import sys
from contextlib import ExitStack

import numpy as np

try:
    import concourse.bass as bass
except ImportError:
    for _p in ("/opt/trn_rl_repo", "/root/.axon_site/_ro/trn_rl_repo"):
        if os.path.isdir(_p):
            sys.path.insert(0, _p)
            break
    import concourse.bass as bass
from concourse import mybir
from concourse.bass_utils import run_bass_kernel_spmd

F32 = mybir.dt.float32
AF = mybir.ActivationFunctionType
ALU = mybir.AluOpType

B_FULL, T_FULL = 1024, 512
N_CORES = 8
BC = B_FULL // N_CORES        # 128 per-core batch
MAGIC = 12582912.0            # 1.5 * 2**23 fp32 round-to-nearest-int trick
TWO_PI = 2.0 * math.pi
R2PI3 = 3.0 / TWO_PI
HALF_PI = 0.5 * math.pi


class Builder:
    """Two-pass program builder: pass 0 records semaphore milestones, pass 1 emits."""

    def __init__(self, T_steps, G=1, no_c=False, no_obs=False, no_cdve=False, no_cmm=False, no_m=False, no_dc=False, offset=2):
        self.no_c = no_c
        self.no_obs = no_obs
        self.no_cdve = no_cdve
        self.no_cmm = no_cmm
        self.no_m = no_m
        self.no_dc = no_dc
        self.offset = offset
        self.T = T_steps
        self.G = G
        self.Bg = BC // G
        self.n_win = (T_steps + 1 + 127) // 128
        self.miles = {}
        self.emit = False

    # ---------- helpers ----------
    def _wait(self, i, wait):
        if wait is not None:
            sem, key = wait
            i._wait_ge(sem, self.miles[key])

    def pe_mm(self, out, lhsT, rhs, start, stop, wait=None, inc=False, mkey=None):
        if inc:
            self.c_pe += 1
        if mkey is not None and not self.emit:
            self.miles[mkey] = self.c_pe
        if self.emit:
            i = self.nc.tensor.matmul(out, lhsT, rhs, start=start, stop=stop)
            self._wait(i, wait)
            if inc:
                i.then_inc(self.s_pe, 1)

    def act_sin(self, out, in_, bias, wait_key, mkey):
        self.c_act += 1
        if not self.emit:
            self.miles[mkey] = self.c_act
        if self.emit:
            i = self.nc.scalar.activation(out, in_, AF.Sin, bias=bias, scale=3.0)
            i._wait_ge(self.s_pe, self.miles[wait_key])
            i.then_inc(self.s_act, 1)

    def dve_op(self, emit_fn, args, wr, rd, wait=None, mkey=None):
        """Emit one DVE op with same-engine RAW/WAR/WAW tracking.

        wr/rd: lists of region keys. Every DVE op increments s_dve; its index
        is the post-inc count. Dependent ops attach a wait on the producing
        index (standalone wait if the attached slot is taken by `wait`).
        """
        self.c_dve += 1
        idx = self.c_dve
        if mkey is not None and not self.emit:
            self.miles[mkey] = idx
        if not self.emit:
            need = 0
            for k in rd:
                need = max(need, self.dve_lastw.get(k, 0))
            for k in wr:
                need = max(need, self.dve_lastw.get(k, 0),
                           self.dve_lastr.get(k, 0))
            if need > self.dve_mark:
                self.dve_extra[idx] = need
                self.dve_mark = need
            for k in rd:
                self.dve_lastr[k] = max(self.dve_lastr.get(k, 0), idx)
            for k in wr:
                self.dve_lastw[k] = idx
                self.dve_lastr[k] = 0
        else:
            extra = self.dve_extra.get(idx)
            if extra is not None and wait is not None:
                self.nc.vector.wait_ge(self.s_dve, extra)
                extra = None
            i = emit_fn(*args)
            if extra is not None:
                i._wait_ge(self.s_dve, extra)
            elif wait is not None:
                sem, key = wait
                i._wait_ge(sem, self.miles[key])
            i.then_inc(self.s_dve, 1)

    def tsp(self, out, in0, s0, s1, op0, op1, wr=(), rd=(), wait=None, mkey=None):
        self.dve_op(self.nc.vector.tensor_scalar,
                    (out, in0, s0, s1, op0, op1), wr, rd, wait=wait, mkey=mkey)

    def tsv(self, out, in0, scal_ap, wr=(), rd=(), wait=None, mkey=None):
        self.dve_op(self.nc.vector.tensor_scalar_mul,
                    (out, in0, scal_ap), wr, rd, wait=wait, mkey=mkey)

    def tt(self, out, in0, in1, op, wr=(), rd=(), wait=None, mkey=None):
        self.dve_op(self.nc.vector.tensor_tensor,
                    (out, in0, in1, op), wr, rd, wait=wait, mkey=mkey)

    def stt(self, out, in0, scal, in1, op0, op1, wr=(), rd=(), wait=None,
            mkey=None):
        self.dve_op(self.nc.vector.scalar_tensor_tensor,
                    (out, in0, scal, in1, op0, op1), wr, rd, wait=wait, mkey=mkey)

    # ---------- engine programs (unit-based, group-offset interleave) ----------
    def _sched(self, unit_fn, flush_fn=None):
        """Drive unit_fn(s, d, g) over all (step, dyn) ticks with group g's
        stream lagging g*OFFSET ticks behind group 0."""
        T, G, OFF = self.T, self.G, self.offset
        total = 4 * T
        for tau in range(total + OFF * (G - 1)):
            for g in range(G):
                tg = tau - OFF * g
                if 0 <= tg < total:
                    s, d = divmod(tg, 4)
                    unit_fn(s, d, g)
                    if flush_fn is not None and g == G - 1:
                        flush_fn(s, d)

    def pe_unit(self, s, d, g):
        Bg = self.Bg
        gc = slice(g * Bg, (g + 1) * Bg)
        par = s % 2
        if d == 0:
            X = self.y[par][:, gc]
            wait = (self.s_dve, ("y", s - 1, g)) if s > 0 else                    (self.s_dve, ("init",))
        else:
            X = [None, self.ya, self.yb, self.yc][d][:, gc]
            wait = (self.s_dve, (("ya", "yb", "yc")[d - 1], s, g))
        U = self.U[g][0:32, 0:Bg]
        # constant mms first (start=True clears the bank): they run during the
        # state wait. Bank-WAR: previous readers are act1(prev dyn) and, for
        # d==1, the m1 measurement op.
        if s == 0 and d == 0:
            uwait = None
        elif d == 1 and not (self.no_c or self.no_cdve):
            uwait = (self.s_dve, ("m1", s, g))
        else:
            pd = (s, d - 1) if d > 0 else (s - 1, 3)
            uwait = (self.s_act, ("h1", pd[0], pd[1], g))
        self.pe_mm(U, self.w1l[:, :], self.latT[:, gc], True, False, wait=uwait)
        if not (self.no_c or self.no_cmm):
            self.pe_mm(U, self.i32[:, :], self.c1[par][:, gc], False, False)
        self.pe_mm(U, self.w1y[:, :], X, False, True, wait=wait,
                   inc=True, mkey=("u", s, d, g))
        if d == 0 and not self.no_obs:
            w, sl = s // 128, s % 128
            wait = None
            if sl == 0 and w > 0:
                wait = (self.s_dve, ("flush", w - 1))
            self.pe_mm(self.OBS[g * Bg:(g + 1) * Bg, 3 * sl:3 * sl + 3],
                       self.y[par][0:64, gc], self.wobs[:, :], True, True,
                       wait=wait, inc=True, mkey=("obsw", w))
        P2 = self.P2[g][0:64, 0:Bg]
        if self.no_c or self.no_cmm:
            self.pe_mm(P2, self.w2[:, :], self.h1[:, gc], True, True,
                       wait=(self.s_act, ("h1", s, d, g)),
                       inc=True, mkey=("p2", s, d, g))
        else:
            if s == 0 and d == 0:
                pwait = None
            elif d == 1 and not self.no_cdve:
                pwait = (self.s_dve, ("m2", s, g))
            else:
                pd = (s, d - 1) if d > 0 else (s - 1, 3)
                pwait = (self.s_act, ("h2", pd[0], pd[1], g))
            self.pe_mm(P2, self.i64[:, :], self.c2[par][:, gc], True, False,
                       wait=pwait)
            self.pe_mm(P2, self.w2[:, :], self.h1[:, gc], False, True,
                       wait=(self.s_act, ("h1", s, d, g)),
                       inc=True, mkey=("p2", s, d, g))
        self.pe_mm(self.Gk[g][0:64, 0:Bg], self.w3a[:, :], self.h2a[:, gc],
                   True, True, wait=(self.s_act, ("h2", s, d, g)),
                   inc=True, mkey=("g", s, d, g))

    def pe_prog(self, pe):
        if self.emit:
            pe.wait_ge(self.dma_in, 16 * self.n_inputs)
        self._sched(self.pe_unit)
        if self.no_obs:
            return
        # final obs slot T (state after the last step)
        T, Bg = self.T, self.Bg
        s = T
        w, sl = s // 128, s % 128
        for g in range(self.G):
            gc = slice(g * Bg, (g + 1) * Bg)
            self.pe_mm(self.OBS[g * Bg:(g + 1) * Bg, 3 * sl:3 * sl + 3],
                       self.y[T % 2][0:64, gc], self.wobs[:, :], True, True,
                       wait=(self.s_dve, ("y", T - 1, g)), inc=True,
                       mkey=("obsw", w))

    def act_unit(self, s, d, g):
        Bg = self.Bg
        gc = slice(g * Bg, (g + 1) * Bg)
        self.act_sin(self.h1[:, gc], self.U[g][0:32, 0:Bg], self.b1v[:, :],
                     ("u", s, d, g), ("h1", s, d, g))
        self.act_sin(self.h2a[0:64, gc], self.P2[g][0:64, 0:Bg], self.b2v[:, :],
                     ("p2", s, d, g), ("h2", s, d, g))

    def act_prog(self, act):
        if self.emit:
            act.wait_ge(self.dma_in, 16 * self.n_inputs)
        self._sched(self.act_unit)

    def flush(self, w):
        ncols = 3 * min(128, self.T + 1 - 128 * w)
        self.dve_op(self.nc.vector.tensor_copy,
                    (self.stage[:, w * 384:w * 384 + ncols],
                     self.OBS[0:128, 0:ncols]),
                    [("stage", w)], [], wait=(self.s_pe, ("obsw", w)),
                    mkey=("flush", w))

    def dve_unit(self, s, d, g):
        G, Bg = self.G, self.Bg
        gc = slice(g * Bg, (g + 1) * Bg)
        par, nxt = s % 2, 1 - s % 2
        ts = slice(s, s + 1)
        y = self.y[par]
        YB = [("y", par, "t", g), ("y", par, "b", g)]
        W = lambda n: [(n, "t", g), (n, "b", g)]
        F = lambda n: [(n, "f", g)]
        cdve = not (self.no_c or self.no_cdve)
        if d == 0:
            self.tsv(self.wc1[0:64, gc], y[64:128, gc], self.tab_a[64:128, ts],
                     wr=[("wc1", "t", g)], rd=[("y", par, "b", g)])
            if cdve:
                # must precede ya: PE's next-psum-write transitivity anchors
                # on the ya milestone
                self.tsp(self.m1[:, gc], self.U[g][0:32, 0:Bg], R2PI3,
                         self.mb1[:, :], ALU.mult, ALU.add,
                         wr=[("m1", g)], rd=[],
                         wait=(self.s_act, ("h1", s, 0, g)),
                         mkey=("m1", s, g))
                self.tsp(self.m2[:, gc], self.P2[g][0:64, 0:Bg], R2PI3,
                         self.mb2[:, :], ALU.mult, ALU.add,
                         wr=[("m2", g)], rd=[],
                         wait=(self.s_act, ("h2", s, 0, g)),
                         mkey=("m2", s, g))
            self.tsv(self.wc1[64:128, gc], self.Gk[g][0:64, 0:Bg],
                     self.tab_a[0:64, ts], wr=[("wc1", "b", g)], rd=[],
                     wait=(self.s_pe, ("g", s, 0, g)))
            self.tt(self.ya[:, gc], y[:, gc], self.wc1[:, gc], ALU.add,
                    wr=W("ya"), rd=YB + W("wc1"), mkey=("ya", s, g))
        elif d == 1:
            if cdve:
                self.tsp(self.d1[:, gc], self.m1[:, gc], MAGIC, -TWO_PI / 3.0,
                         ALU.subtract, ALU.mult, wr=[("d1", g)], rd=[("m1", g)])
                self.tt(self.c1[nxt][:, gc], self.c1[par][:, gc],
                        self.d1[:, gc], ALU.add, wr=[("c1", nxt, g)],
                        rd=[("c1", par, g), ("d1", g)])
            self.tsv(self.wc2[0:64, gc], self.ya[64:128, gc],
                     self.tab_d[64:128, ts],
                     wr=[("wc2", "t", g)], rd=[("ya", "b", g)])
            self.tt(self.mt[:, gc], y[:, gc], self.wc1[:, gc], ALU.subtract,
                    wr=F("mt"), rd=YB + W("wc1"))
            self.stt(self.nt[:, gc], self.wc1[:, gc], 3.0, y[:, gc],
                     ALU.mult, ALU.add, wr=F("nt"), rd=W("wc1") + YB)
            self.tsv(self.wc2[64:128, gc], self.Gk[g][0:64, 0:Bg],
                     self.tab_d[0:64, ts], wr=[("wc2", "b", g)], rd=[],
                     wait=(self.s_pe, ("g", s, 1, g)))
            self.tt(self.yb[:, gc], self.mt[:, gc], self.wc2[:, gc], ALU.add,
                    wr=W("yb"), rd=F("mt") + W("wc2"), mkey=("yb", s, g))
        elif d == 2:
            if cdve:
                self.tsp(self.d2[:, gc], self.m2[:, gc], MAGIC, -TWO_PI / 3.0,
                         ALU.subtract, ALU.mult, wr=[("d2", g)], rd=[("m2", g)])
                self.tt(self.c2[nxt][:, gc], self.c2[par][:, gc],
                        self.d2[:, gc], ALU.add, wr=[("c2", nxt, g)],
                        rd=[("c2", par, g), ("d2", g)])
            self.tsv(self.wc3[0:64, gc], self.yb[64:128, gc],
                     self.tab_d[64:128, ts],
                     wr=[("wc3", "t", g)], rd=[("yb", "b", g)])
            self.tt(self.n2t[:, gc], self.nt[:, gc], self.wc2[:, gc],
                    ALU.subtract, wr=F("n2t"), rd=F("nt") + W("wc2"))
            self.tt(self.rt[:, gc], self.wc1[:, gc], self.wc2[:, gc], ALU.add,
                    wr=F("rt"), rd=W("wc1") + W("wc2"))
            self.tsv(self.wc3[64:128, gc], self.Gk[g][0:64, 0:Bg],
                     self.tab_d[0:64, ts], wr=[("wc3", "b", g)], rd=[],
                     wait=(self.s_pe, ("g", s, 2, g)))
            self.tt(self.yc[:, gc], self.n2t[:, gc], self.wc3[:, gc], ALU.add,
                    wr=W("yc"), rd=F("n2t") + W("wc3"), mkey=("yc", s, g))
        else:
            self.tt(self.r2t[:, gc], self.rt[:, gc], self.wc3[:, gc], ALU.add,
                    wr=F("r2t"), rd=F("rt") + W("wc3"))
            self.stt(self.zt[:, gc], self.r2t[:, gc], 0.375, y[:, gc],
                     ALU.mult, ALU.add, wr=F("zt"), rd=F("r2t") + YB)
            self.tsv(self.wc4[0:64, gc], self.yc[64:128, gc],
                     self.tab_e[64:128, ts],
                     wr=[("wc4", "t", g)], rd=[("yc", "b", g)])
            self.tsv(self.wc4[64:128, gc], self.Gk[g][0:64, 0:Bg],
                     self.tab_e[0:64, ts], wr=[("wc4", "b", g)], rd=[],
                     wait=(self.s_pe, ("g", s, 3, g)))
            self.tt(self.y[nxt][:, gc], self.zt[:, gc], self.wc4[:, gc],
                    ALU.add, wr=[("y", nxt, "t", g), ("y", nxt, "b", g)],
                    rd=F("zt") + W("wc4"), mkey=("y", s, g))

    def _dve_flush_check(self, s, d):
        if self.no_obs or d != 0:
            return
        # window w is complete once every group's obs slot 128w+127 has run;
        # slot s is emitted in tick (s, d=0), so flush after the last group's
        # (s=128w+127, 0) tick -- but obs for slot s happens at d==0 of step s,
        # and this callback fires after group G-1's (s, 0) unit.
        if s % 128 == 127 and (s // 128) < self.n_win - 1:
            self.flush(s // 128)

    def dve_prog(self, dve):
        if self.emit:
            dve.wait_ge(self.dma_in, 16 * self.n_inputs)
        self.c_dve += 1
        if not self.emit:
            self.miles[("init",)] = self.c_dve
        if self.emit:
            self.nc.vector.memset(self.h2a[64:65, :], 1.0).then_inc(self.s_dve, 1)
        self._sched(self.dve_unit, self._dve_flush_check)
        if not self.no_obs:
            self.flush(self.n_win - 1)

    def sp_prog(self, sp):
        if not self.emit:
            return
        for name in self.in_order:
            sp.dma_start(out=self.in_tiles[name][:, :],
                         in_=self.dram_in[name][:, :]).then_inc(self.dma_in, 16)
        if self.no_obs:
            sp.wait_ge(self.s_dve, self.miles[("y", self.T - 1, 0)])
            sp.dma_start(out=self.dram_out[:, 0:128],
                         in_=self.y[self.T % 2][:, :]).then_inc(self.dma_in, 16)
            return
        sp.wait_ge(self.s_dve, self.miles[("flush", self.n_win - 1)])
        nct = 384 * (self.n_win - 1) + 3 * min(128, self.T + 1 - 128 * (self.n_win - 1))
        sp.dma_start(out=self.dram_out[:, :nct],
                     in_=self.stage[:, :nct]).then_inc(self.dma_in, 16)

    # ---------- build ----------
    def build(self):
        nc = bass.Bass("TRN2", target_bir_lowering=False, debug=False,
                       num_devices=N_CORES)
        self.nc = nc
        T, G = self.T, self.G
        es = ExitStack()
        self.es = es
        self.in_tiles = {}
        self.dram_in = {}
        self.in_order = []

        def inp(name, shape):
            self.dram_in[name] = nc.dram_tensor(name, shape, F32,
                                                kind="ExternalInput").ap()
            tile = es.enter_context(nc.sbuf_tensor(name + "_sb", shape, F32))
            self.in_tiles[name] = tile
            self.in_order.append(name)
            return tile

        def sb(name, shape):
            return es.enter_context(nc.sbuf_tensor(name, shape, F32))

        self.y = [inp("y0", [128, BC]), sb("y1_sb", [128, BC])]
        self.latT = inp("latT", [128, BC])
        self.w1y = inp("w1y", [128, 32])
        self.w1l = inp("w1l", [128, 32])
        self.w2 = inp("w2", [32, 64])
        self.w3a = inp("w3a", [65, 64])
        self.wobs = inp("wobs", [64, 3])
        self.b1v = inp("b1v", [32, 1])
        self.b2v = inp("b2v", [64, 1])
        self.mb1 = inp("mb1", [32, 1])
        self.mb2 = inp("mb2", [64, 1])
        self.i32 = inp("i32", [32, 32])
        self.i64 = inp("i64", [64, 64])
        self.c1 = [inp("c1i", [32, BC]), sb("c1b_sb", [32, BC])]
        self.c2 = [inp("c2i", [64, BC]), sb("c2b_sb", [64, BC])]
        self.tab_a = inp("tab_a", [128, max(T, 1)])
        self.tab_d = inp("tab_d", [128, max(T, 1)])
        self.tab_e = inp("tab_e", [128, max(T, 1)])
        self.n_inputs = len(self.in_order)

        self.h1 = sb("h1", [32, BC])
        self.h2a = sb("h2a", [65, BC])
        self.ya = sb("ya", [128, BC])
        self.yb = sb("yb", [128, BC])
        self.yc = sb("yc", [128, BC])
        self.wc1 = sb("wc1", [128, BC])
        self.wc2 = sb("wc2", [128, BC])
        self.wc3 = sb("wc3", [128, BC])
        self.wc4 = sb("wc4", [128, BC])
        self.mt = sb("mt", [128, BC])
        self.nt = sb("nt", [128, BC])
        self.n2t = sb("n2t", [128, BC])
        self.rt = sb("rt", [128, BC])
        self.r2t = sb("r2t", [128, BC])
        self.zt = sb("zt", [128, BC])
        self.m1 = sb("m1", [32, BC])
        self.m2 = sb("m2", [64, BC])
        self.d1 = sb("d1", [32, BC])
        self.d2 = sb("d2", [64, BC])
        self.stage = sb("stage", [128, 384 * self.n_win])

        self.U = [es.enter_context(nc.psum_tensor(f"U{g}", [32, 512], F32))
                  for g in range(G)]
        self.P2 = [es.enter_context(nc.psum_tensor(f"P2{g}", [64, 512], F32))
                   for g in range(G)]
        self.Gk = [es.enter_context(nc.psum_tensor(f"Gk{g}", [64, 512], F32))
                   for g in range(G)]
        self.OBS = es.enter_context(nc.psum_tensor("OBS", [128, 384], F32))

        self.dram_out = nc.dram_tensor("ych", [128, 384 * self.n_win], F32,
                                       kind="ExternalOutput").ap()

        self.dma_in = es.enter_context(nc.semaphore("dma_in"))
        self.s_pe = es.enter_context(nc.semaphore("s_pe"))
        self.s_act = es.enter_context(nc.semaphore("s_act"))
        self.s_dve = es.enter_context(nc.semaphore("s_dve"))

        # pass 0: record milestones
        self.emit = False
        self.c_pe = self.c_act = self.c_dve = 0
        self.dve_lastw = {}
        self.dve_lastr = {}
        self.dve_mark = 0
        self.dve_extra = {}
        self.pe_prog(None)
        self.act_prog(None)
        self.dve_prog(None)

        # pass 1: emit
        self.emit = True
        self.c_pe = self.c_act = self.c_dve = 0
        block = es.enter_context(nc.Block())

        @block.tensor
        def _(pe):
            self.pe_prog(pe)

        @block.scalar
        def _(act):
            self.act_prog(act)

        @block.vector
        def _(dve):
            self.dve_prog(dve)

        @block.sync
        def _(sp):
            self.sp_prog(sp)

        return nc


# ---------------- host side ----------------

def make_core_inputs(latent, t, W_l2d, b_l2d, W1, b1, W2, b2, W3, b3,
                     W_obs, b_obs, T_steps):
    """Host preprocessing -> list of per-core input dicts (numpy fp32)."""
    f = np.float32
    latent = np.asarray(latent, f)
    t = np.asarray(t, f)
    y0 = latent @ np.asarray(W_l2d, f) + np.asarray(b_l2d, f)   # [B, 128]
    dts = (t[1:] - t[:-1]).astype(f)[:T_steps]
    T = max(T_steps, 1)
    tab = np.zeros((3, 128, T), f)
    if T_steps > 0:
        tab[0, :, :T_steps] = (dts / 3.0).astype(f)[None, :]
        tab[1, :, :T_steps] = dts[None, :]
        tab[2, :, :T_steps] = (dts / 8.0).astype(f)[None, :]
    W1 = np.asarray(W1, f)
    b1v = (3.0 * np.asarray(b1, f) + HALF_PI).astype(f).reshape(32, 1)
    b2v = (3.0 * np.asarray(b2, f) + HALF_PI).astype(f).reshape(64, 1)
    mb1 = (MAGIC + b1v / TWO_PI).astype(f)
    mb2 = (MAGIC + b2v / TWO_PI).astype(f)
    w3a = np.vstack([np.asarray(W3, f), np.asarray(b3, f)[None, :]]).astype(f)
    i32 = np.eye(32, dtype=f)
    i64 = np.eye(64, dtype=f)

    cores = []
    for c in range(N_CORES):
        bs = slice(c * BC, (c + 1) * BC)
        latT = np.ascontiguousarray(latent[bs].T)        # [128, BC]
        y0T = np.ascontiguousarray(y0[bs].T)             # [128, BC]
        # bootstrap angle corrections from y0 (u1/p2 of dyn1 at step 0)
        u1 = W1[0:128].T @ y0T + W1[128:256].T @ latT    # [32, BC]
        n1 = np.rint((3.0 * u1 + b1v) / TWO_PI).astype(f)
        c1i = (-(TWO_PI / 3.0) * n1).astype(f)
        h1 = np.cos(3.0 * (u1 + np.asarray(b1, f)[:, None])).astype(f)
        p2 = np.asarray(W2, f).T @ h1                    # [64, BC]
        n2 = np.rint((3.0 * p2 + b2v) / TWO_PI).astype(f)
        c2i = (-(TWO_PI / 3.0) * n2).astype(f)
        cores.append({
            "y0": y0T, "latT": latT,
            "w1y": np.ascontiguousarray(W1[0:128]),
            "w1l": np.ascontiguousarray(W1[128:256]),
            "w2": np.asarray(W2, f), "w3a": w3a,
            "wobs": np.asarray(W_obs, f),
            "b1v": b1v, "b2v": b2v, "mb1": mb1, "mb2": mb2,
            "i32": i32, "i64": i64,
            "c1i": c1i, "c2i": c2i,
            "tab_a": tab[0], "tab_d": tab[1], "tab_e": tab[2],
        })
    return cores


_CACHE = {}


def get_nc(T_steps, G=1, **bkw):
    key = (T_steps, G, tuple(sorted(bkw.items())))
    if key not in _CACHE:
        _CACHE[key] = Builder(T_steps, G, **bkw).build()
    return _CACHE[key]


def run_device(inputs_by_core, T_steps, G=1, bkw=None, **kw):
    nc = get_nc(T_steps, G, **(bkw or {}))
    return run_bass_kernel_spmd(nc, inputs_by_core, list(range(N_CORES)), **kw)


def kernel(latent, t, W_l2d, b_l2d, W1, b1, W2, b2, W3, b3, W_obs, b_obs):
    T_steps = T_FULL - 1
    cores = make_core_inputs(latent, t, W_l2d, b_l2d, W1, b1, W2, b2, W3, b3,
                             W_obs, b_obs, T_steps)
    res = run_device(cores, T_steps)
    n_win = (T_steps + 1 + 127) // 128
    f = np.float32
    b_obs = np.asarray(b_obs, f)
    traj = np.empty((T_FULL, B_FULL, 3), f)
    for c in range(N_CORES):
        ych = res.results[c]["ych"].reshape(BC, n_win * 128, 3)[:, :T_FULL, :]
        traj[:, c * BC:(c + 1) * BC, :] = np.swapaxes(ych, 0, 1)
    traj += b_obs[None, None, :]
    traj_x = np.ascontiguousarray(traj[:, :, :2])
    traj_p = 1.0 / (1.0 + np.exp(-traj[:, :, 2], dtype=f))
    logpt = np.zeros((T_FULL, B_FULL, 1), f)
    return traj_x, traj_p.astype(f), logpt
